# revision 6
# baseline (speedup 1.0000x reference)
"""CBAM-style channel+spatial attention block (nn_CBAModule) on 8 TRN2
NeuronCores, pure data-parallel over the batch dimension.

Self-contained: builds a Bass/Tile kernel per 4-image shard, runs it SPMD on
cores 0-7 via concourse.bass_utils.run_bass_kernel_spmd, gathers the full
outputs.  All weight reshaping/transposition is done host-side in numpy and
fed as extra kernel inputs.

The pinned walrus codegen accepts at most 1 sync-wait on DMA
pseudo-instructions and 2 on everything else, while Tile emits transitively
redundant waits; fix_waits() below prunes them (see its docstring for the
soundness argument).
"""

import sys

for _p in ("/opt/trn_rl_repo",):
    if _p not in sys.path:
        sys.path.insert(0, _p)

import numpy as np

import concourse.tile_sem_assignment as _tsa

_pinned = False


def pin_dma_lanes():
    """Force every DMA onto one Tile completion-sem lane (must run before
    TileContext is created).  All DMAs are issued from the single SP HWDGE
    ring, so single-lane cumulative thresholds stay sound: every DMA adds
    exactly 16 increments, hence observing lane >= 16*k where k DMAs were
    issued earlier guarantees all k completed regardless of completion
    order."""
    global _pinned
    if _pinned:
        return
    _pinned = True
    orig = _tsa.TileClockTick.__init__

    def patched(self, *a, **k):
        orig(self, *a, **k)
        self.swdge_sem_count = 1

    _tsa.TileClockTick.__init__ = patched
    _tsa.TileClockTick.next_hw_dma_idx = property(
        lambda self: 0, lambda self, v: None
    )


_SERIAL_ENGINES = {"EngineType.DVE", "EngineType.Activation", "EngineType.Pool"}


def _join(a, b):
    for k, v in b.items():
        if a.get(k, -1) < v:
            a[k] = v
    return a


def fix_waits(nc, dma_deps=None, verbose=False):
    """Drop sync waits provably implied transitively and lower/drop the Tile
    DMA-lane serialization waits using the kernel's true-DMA-dependency
    annotations; assert the walrus per-instruction wait limits.

    Assumptions: DVE/ACT/Pool retire instruction N before dispatching N+1
    (drain semantics); an instruction dispatches only after its waits held
    and its same-engine predecessor dispatched; a DMA's remaining hazards
    are covered by its other waits when its lane wait is lowered to its
    annotated DMA-dependencies."""
    import concourse.mybir as mybir

    dma_deps = dma_deps or {}
    n_dropped = n_lowered = 0
    violations = []
    for fn in nc.m.functions:
        insts = []
        for blk in fn.blocks:
            insts.extend(blk.instructions)
        bad_sems = set()
        for inst in insts:
            si = inst.sync_info
            if si is None:
                continue
            for u in si.on_update:
                if u.update_mode not in ("sem-inc", "sem-add-imm"):
                    bad_sems.add(u.ant_name)
            for w in si.on_wait:
                if w.wait_mode != "sem-ge-imm":
                    bad_sems.add(w.ant_name)

        lane_cum = {}
        lane_counter = {}
        for inst in insts:
            si = inst.sync_info
            if si is None or "DMA" not in type(inst).__name__:
                continue
            for u in si.on_update:
                if u.update_mode in ("sem-inc", "sem-add-imm") and (
                    u.ant_name.startswith("DMAHW")
                    or u.ant_name.startswith("DMASW")
                ):
                    c = lane_counter.get(u.ant_name, 0) + (u.update_value or 16)
                    lane_counter[u.ant_name] = c
                    lane_cum[inst.name] = (u.ant_name, c)

        sem_hist = {}
        eng_know = {}
        eng_count = {}

        def lookup(sem, v):
            k = {sem: v}
            for cum, clk in sem_hist.get(sem, []):
                _join(k, clk)
                if cum >= v:
                    break
            return k

        for inst in insts:
            si = inst.sync_info
            tname = type(inst).__name__
            eng = str(getattr(inst, "engine", None))
            k0 = dict(eng_know.get(eng, {}))
            if eng in _SERIAL_ENGINES:
                for s, c in eng_count.items():
                    if s.startswith(eng):
                        _join(k0, {s.split("|", 1)[1]: c})
            if si is None:
                eng_know[eng] = k0
                continue
            waits = list(si.on_wait)
            is_dma = "DMA" in tname

            lowered = False
            if is_dma and inst.name in lane_cum:
                lane_sem, _my_cum = lane_cum[inst.name]
                need = 0
                for dep in dma_deps.get(inst.name, ()):
                    if dep not in lane_cum:
                        raise RuntimeError(
                            f"waitfix: unknown dma dep {dep} of {inst.name}"
                        )
                    dsem, dcum = lane_cum[dep]
                    if dsem != lane_sem:
                        raise RuntimeError(
                            f"waitfix: cross-lane dep {dep}->{inst.name}"
                        )
                    need = max(need, dcum)
                for i, w in enumerate(waits):
                    if w.ant_name == lane_sem and w.wait_mode == "sem-ge-imm":
                        if need == 0:
                            waits = waits[:i] + waits[i + 1 :]
                            lowered = True
                        elif w.wait_value > need:
                            nw = mybir.SyncWait(
                                sync_type=w.sync_type,
                                id=w.id,
                                ant_name=w.ant_name,
                                wait_mode=w.wait_mode,
                                wait_value=need,
                                wait_reg=w.wait_reg,
                            )
                            waits = waits[:i] + [nw] + waits[i + 1 :]
                            lowered = True
                        break
                if lowered:
                    n_lowered += 1

            looks = {}
            for i, w in enumerate(waits):
                if w.wait_mode == "sem-ge-imm" and w.ant_name not in bad_sems:
                    looks[i] = lookup(w.ant_name, w.wait_value)
            kept = set(range(len(waits)))
            changed = True
            while changed:
                changed = False
                for i in sorted(kept):
                    if i not in looks:
                        continue
                    w = waits[i]
                    cl = dict(k0)
                    for j in kept:
                        if j == i or j not in looks:
                            continue
                        _join(cl, looks[j])
                    if cl.get(w.ant_name, -1) >= w.wait_value:
                        kept.discard(i)
                        changed = True
            kd = k0
            for i, lk in looks.items():
                _join(kd, lk)
            eng_know[eng] = kd

            for u in si.on_update:
                if u.update_mode in ("sem-inc", "sem-add-imm"):
                    s = u.ant_name
                    if s in bad_sems:
                        continue
                    hist = sem_hist.setdefault(s, [])
                    prev = hist[-1][0] if hist else 0
                    hist.append((prev + (u.update_value or 1), dict(kd)))
                    if not (s.startswith("DMAHW") or s.startswith("DMASW")):
                        key = f"{eng}|{s}"
                        eng_count[key] = eng_count.get(key, 0) + (
                            u.update_value or 1
                        )

            if len(kept) < len(waits) or lowered:
                n_dropped += len(waits) - len(kept)
                new_waits = [waits[i] for i in sorted(kept)]
                inst.sync_info = mybir.SyncInfo(
                    on_wait=new_waits, on_update=list(si.on_update)
                )
            limit = 1 if is_dma else 2
            if len(kept) > limit:
                violations.append(
                    (
                        inst.name,
                        tname,
                        eng,
                        [
                            (waits[i].ant_name, waits[i].wait_value)
                            for i in sorted(kept)
                        ],
                    )
                )
    if verbose:
        print(f"waitfix: dropped {n_dropped}, lowered {n_lowered} waits")
        for v in violations:
            print("  VIOLATION:", v)
    if violations:
        raise RuntimeError(
            f"waitfix: {len(violations)} instructions exceed walrus sync-wait "
            f"limits: {violations[:8]}"
        )


B, C, H, W = 32, 512, 56, 56
S = H * W
G = C // 128
CR = 32
N_CORES = 8
B_LOC = B // N_CORES
Q = 112
NJ = S // Q  # 28 pixel chunks of 112 (3136 = 28*112)


def build_nc(repeat=1):
    pin_dma_lanes()
    import concourse.bass as bass
    import concourse.mybir as mybir
    import concourse.tile as tile

    f32 = mybir.dt.float32
    ts = bass.ts
    AF = mybir.ActivationFunctionType
    OP = mybir.AluOpType

    nc = bass.Bass("TRN2")
    x_in = nc.dram_tensor("x", [B_LOC, C, S], f32, kind="ExternalInput")
    w1t_avg = nc.dram_tensor("w1t_avg", [128, G, CR], f32, kind="ExternalInput")
    w1t_max = nc.dram_tensor("w1t_max", [128, G, CR], f32, kind="ExternalInput")
    w2t = nc.dram_tensor("w2t", [CR, G, 128], f32, kind="ExternalInput")
    b1d = nc.dram_tensor("b1c", [CR, 1], f32, kind="ExternalInput")
    b2x2 = nc.dram_tensor("b2x2", [128, G], f32, kind="ExternalInput")
    wbd = nc.dram_tensor("wb", [60, 10, 56], f32, kind="ExternalInput")
    b3d = nc.dram_tensor("b3c", [56, 1], f32, kind="ExternalInput")
    onesd = nc.dram_tensor("ones", [128, 1], f32, kind="ExternalInput")
    identd = nc.dram_tensor("ident", [128, 128], f32, kind="ExternalInput")

    out_d = nc.dram_tensor("out", [B_LOC, C, S], f32, kind="ExternalOutput")
    ca_d = nc.dram_tensor("ca", [B_LOC, C], f32, kind="ExternalOutput")
    sp_d = nc.dram_tensor("sp", [B_LOC, S], f32, kind="ExternalOutput")

    mrt = nc.dram_tensor("mrt", [2, 2, S], f32, kind="Internal")

    dma_deps = {}

    def dma(dst, src, deps=()):
        inst = nc.sync.dma_start(out=dst, in_=src)
        dma_deps[inst.ins.name] = [d for d in deps if d is not None]
        return inst.ins.name

    with tile.TileContext(nc) as tc:
        with (
            tc.tile_pool(name="consts", bufs=1) as consts,
            tc.tile_pool(name="big", bufs=1) as big,
            tc.tile_pool(name="work", bufs=1) as work,
            tc.tile_pool(name="ps", bufs=1, space="PSUM") as psp,
        ):
            # ---- constants ----
            w1a_sb = consts.tile([128, G, CR], f32, name="w1a_sb")
            w1m_sb = consts.tile([128, G, CR], f32, name="w1m_sb")
            w2t_sb = consts.tile([CR, G, 128], f32, name="w2t_sb")
            b1_sb = consts.tile([CR, 1], f32, name="b1_sb")
            b2_sb = consts.tile([128, G], f32, name="b2_sb")
            wb_sb = consts.tile([60, 10, 56], f32, name="wb_sb")
            b3_sb = consts.tile([56, 1], f32, name="b3_sb")
            ones_sb = consts.tile([128, 1], f32, name="ones_sb")
            id_sb = consts.tile([128, 128], f32, name="id_sb")
            caT = consts.tile([128, G, B_LOC], f32, name="caT")
            for dst, src in (
                (w1a_sb, w1t_avg),
                (w1m_sb, w1t_max),
                (w2t_sb, w2t),
                (b1_sb, b1d),
                (b2_sb, b2x2),
                (wb_sb, wbd),
                (b3_sb, b3d),
                (ones_sb, onesd),
                (id_sb, identd),
            ):
                dma(dst, src[:])

            # ---- per-parity working tiles ----
            xx = [big.tile([128, G, S], f32, name=f"xx{p}") for p in range(2)]
            mt1 = [big.tile([128, S], f32, name=f"mt1_{p}") for p in range(2)]
            mt2 = [big.tile([128, S], f32, name=f"mt2_{p}") for p in range(2)]
            spb = [big.tile([128, S], f32, name=f"spb{p}") for p in range(2)]
            pooled = [work.tile([128, G, 2], f32, name=f"pooled{p}") for p in range(2)]
            h_sb = [work.tile([CR, 2], f32, name=f"h_sb{p}") for p in range(2)]
            hsum = [work.tile([CR, 1], f32, name=f"hsum{p}") for p in range(2)]
            pixmax = [work.tile([Q, NJ], f32, name=f"pixmax{p}") for p in range(2)]
            pm_sb = [work.tile([Q, NJ], f32, name=f"pm_sb{p}") for p in range(2)]
            mpad = [work.tile([60, 2, 60], f32, name=f"mpad{p}") for p in range(2)]
            sp_sb = [work.tile([56, 56], f32, name=f"sp_sb{p}") for p in range(2)]
            for p in range(2):
                nc.vector.memset(mpad[p], 0.0)

            st_out = {}
            ld_pad = {}

            for it in range(B_LOC * repeat):
                b = it % B_LOC
                p = it % 2
                xr = x_in[b].rearrange("(g q) s -> q g s", q=128)
                ld_x = dma(xx[p], xr, deps=[st_out.get(it - 2)])

                # ---- pooled stats ----
                for g in range(G):
                    nc.vector.reduce_sum(
                        out=pooled[p][:, g, 0:1], in_=xx[p][:, g, :],
                        axis=mybir.AxisListType.X,
                    )
                    nc.vector.reduce_max(
                        out=pooled[p][:, g, 1:2], in_=xx[p][:, g, :],
                        axis=mybir.AxisListType.X,
                    )

                # ---- MLP ----
                ps_h = psp.tile([CR, 2], f32, name=f"ps_h_{it}", tag="ps_mlp", bufs=2)
                for col, wt in ((0, w1a_sb), (1, w1m_sb)):
                    for g in range(G):
                        nc.tensor.matmul(
                            ps_h[:, col : col + 1],
                            wt[:, g, :],
                            pooled[p][:, g, col : col + 1],
                            start=(g == 0),
                            stop=(g == G - 1),
                        )
                nc.scalar.activation(
                    out=h_sb[p], in_=ps_h, func=AF.Relu, bias=b1_sb, scale=1.0
                )
                nc.vector.tensor_tensor(
                    out=hsum[p], in0=h_sb[p][:, 0:1], in1=h_sb[p][:, 1:2],
                    op=OP.add,
                )
                ps_ca = psp.tile([128, G], f32, name=f"ps_ca_{it}", tag="ps_mlp", bufs=2)
                for g in range(G):
                    nc.tensor.matmul(
                        ps_ca[:, g : g + 1], w2t_sb[:, g, :], hsum[p],
                        start=True, stop=True,
                    )
                for g in range(G):
                    nc.scalar.activation(
                        out=caT[:, g, b : b + 1], in_=ps_ca[:, g : g + 1],
                        func=AF.Sigmoid, bias=b2_sb[:, g : g + 1], scale=1.0,
                    )

                # ---- xg = x * ca (in place) ----
                for g in range(G):
                    nc.vector.tensor_scalar_mul(
                        out=xx[p][:, g, :], in0=xx[p][:, g, :],
                        scalar1=caT[:, g, b : b + 1],
                    )

                # ---- channel mean map (PE): psum[q, j] = sum_c xg[c, j*128+q] ----
                ps_pm = psp.tile([Q, NJ], f32, name=f"ps_pm_{it}", tag="ps_pmcv", bufs=2)
                for g in range(G):
                    for j in range(NJ):
                        nc.tensor.matmul(
                            ps_pm[:, j : j + 1],
                            xx[p][:, g, ts(j, Q)],
                            ones_sb,
                            start=(g == 0),
                            stop=(g == G - 1),
                        )
                nc.vector.tensor_copy(out=pm_sb[p], in_=ps_pm)

                # ---- channel max map: group tree then PE transpose + reduce ----
                nc.vector.tensor_tensor(
                    out=mt1[p], in0=xx[p][:, 0, :], in1=xx[p][:, 1, :], op=OP.max
                )
                nc.vector.tensor_tensor(
                    out=mt2[p], in0=xx[p][:, 2, :], in1=xx[p][:, 3, :], op=OP.max
                )
                nc.vector.tensor_tensor(
                    out=mt1[p], in0=mt1[p], in1=mt2[p], op=OP.max
                )
                for jj in range((NJ + 3) // 4):
                    qc = min(4, NJ - jj * 4)
                    ps_tr = psp.tile(
                        [Q, 4, 128], f32, name=f"ps_tr_{it}_{jj}", tag="ps_tr",
                        bufs=2,
                    )
                    for q in range(qc):
                        nc.tensor.transpose(
                            ps_tr[:, q, :], mt1[p][:, ts(jj * 4 + q, Q)], id_sb
                        )
                    nc.vector.reduce_max(
                        out=pixmax[p][:, jj * 4 : jj * 4 + qc],
                        in_=ps_tr[:, :qc, :],
                        axis=mybir.AxisListType.X,
                    )

                # ---- maps -> DRAM -> padded tile ----
                st_m = dma(
                    mrt[p, 0].rearrange("(j q) -> q j", j=NJ, q=Q), pm_sb[p],
                    deps=[ld_pad.get(it - 2)],
                )
                st_x = dma(
                    mrt[p, 1].rearrange("(j q) -> q j", j=NJ, q=Q), pixmax[p],
                    deps=[ld_pad.get(it - 2)],
                )
                ld_pad[it] = dma(
                    mpad[p][2:58, :, 2:58],
                    mrt[p].rearrange("c (h w) -> h c w", w=56),
                    deps=[st_m, st_x],
                )

                # ---- 5x5 conv via banded matmuls ----
                ps_cv = psp.tile([56, 56], f32, name=f"ps_cv_{it}", tag="ps_pmcv", bufs=2)
                first = True
                for c in range(2):
                    for dx in range(5):
                        nc.tensor.matmul(
                            ps_cv,
                            wb_sb[:, c * 5 + dx, :],
                            mpad[p][:, c, dx : dx + 56],
                            start=first,
                            stop=(c == 1 and dx == 4),
                        )
                        first = False
                nc.scalar.activation(
                    out=sp_sb[p], in_=ps_cv, func=AF.Sigmoid, bias=b3_sb, scale=1.0
                )
                st_sp = dma(
                    sp_d[b].rearrange("(h w) -> h w", w=56), sp_sb[p], deps=[]
                )
                ld_spb = dma(
                    spb[p], sp_d[b].partition_broadcast(128), deps=[st_sp]
                )

                # ---- out = xg * sp ----
                for g in range(G):
                    nc.vector.tensor_tensor(
                        out=xx[p][:, g, :], in0=xx[p][:, g, :], in1=spb[p],
                        op=OP.mult,
                    )
                st_out[it] = dma(
                    out_d[b].rearrange("(g q) s -> q g s", q=128), xx[p], deps=[]
                )

            for g in range(G):
                dma(
                    ca_d[:, g * 128 : (g + 1) * 128].rearrange("b q -> q b"),
                    caT[:, g, :],
                    deps=[],
                )

    fix_waits(nc, dma_deps, verbose=False)
    return nc


def _host_prep(w1, b1, w2, b2, w3, b3):
    w1 = np.asarray(w1, np.float32)
    w2 = np.asarray(w2, np.float32)
    w3 = np.asarray(w3, np.float32)
    w1t = np.transpose(w1).reshape(G, 128, CR).transpose(1, 0, 2).copy()
    w1t_avg = (w1t / float(S)).astype(np.float32)
    w1t_max = w1t.astype(np.float32)
    w2t = np.transpose(w2).reshape(CR, G, 128).copy().astype(np.float32)
    b1c = np.asarray(b1, np.float32).reshape(CR, 1).copy()
    b2x2 = (2.0 * np.asarray(b2, np.float32)).reshape(G, 128).T.copy()
    wb = np.zeros((60, 10, 56), np.float32)
    for c in range(2):
        scale = (1.0 / C) if c == 0 else 1.0
        for dx in range(5):
            for dy in range(5):
                for h in range(56):
                    wb[h + dy, c * 5 + dx, h] = w3[0, c, dy, dx] * scale
    b3c = np.full((56, 1), np.asarray(b3, np.float32).reshape(-1)[0], np.float32)
    ones = np.ones((128, 1), np.float32)
    ident = np.eye(128, dtype=np.float32)
    return dict(
        w1t_avg=w1t_avg, w1t_max=w1t_max, w2t=w2t, b1c=b1c, b2x2=b2x2,
        wb=wb, b3c=b3c, ones=ones, ident=ident,
    )


_cached_nc = None


def kernel(x, w1, b1, w2, b2, w3, b3):
    global _cached_nc
    from concourse.bass_utils import run_bass_kernel_spmd

    x = np.ascontiguousarray(np.asarray(x, np.float32))
    params = _host_prep(w1, b1, w2, b2, w3, b3)
    if _cached_nc is None:
        _cached_nc = build_nc()
    nc = _cached_nc
    in_maps = []
    for i in range(N_CORES):
        m = {"x": x[i * B_LOC : (i + 1) * B_LOC].reshape(B_LOC, C, S)}
        m.update(params)
        in_maps.append(m)
    res = run_bass_kernel_spmd(nc, in_maps, core_ids=list(range(N_CORES)))
    out = np.concatenate(
        [r["out"].reshape(B_LOC, C, H, W) for r in res.results], axis=0
    )
    ca = np.concatenate(
        [r["ca"].reshape(B_LOC, C, 1, 1) for r in res.results], axis=0
    )
    sp = np.concatenate(
        [r["sp"].reshape(B_LOC, 1, H, W) for r in res.results], axis=0
    )
    return (out, (ca, sp))


# revision 7
# speedup vs baseline: 2.4659x; 2.4659x over previous
"""CBAM-style channel+spatial attention block (nn_CBAModule) on 8 TRN2
NeuronCores, pure data-parallel over the batch dimension.

Self-contained: builds a Bass/Tile kernel per 4-image shard, runs it SPMD on
cores 0-7 via concourse.bass_utils.run_bass_kernel_spmd, gathers the full
outputs.  All weight reshaping/transposition is done host-side in numpy and
fed as extra kernel inputs.

The pinned walrus codegen accepts at most 1 sync-wait on DMA
pseudo-instructions and 2 on everything else, while Tile emits transitively
redundant waits; fix_waits() below prunes them (see its docstring for the
soundness argument).
"""

import sys

for _p in ("/opt/trn_rl_repo",):
    if _p not in sys.path:
        sys.path.insert(0, _p)

import numpy as np

import concourse.tile_sem_assignment as _tsa

_pinned = False


def pin_dma_lanes():
    """Force every DMA onto one Tile completion-sem lane (must run before
    TileContext is created).  All DMAs are issued from the single SP HWDGE
    ring, so single-lane cumulative thresholds stay sound: every DMA adds
    exactly 16 increments, hence observing lane >= 16*k where k DMAs were
    issued earlier guarantees all k completed regardless of completion
    order."""
    global _pinned
    if _pinned:
        return
    _pinned = True
    orig = _tsa.TileClockTick.__init__

    def patched(self, *a, **k):
        orig(self, *a, **k)
        self.swdge_sem_count = 1

    _tsa.TileClockTick.__init__ = patched
    _tsa.TileClockTick.next_hw_dma_idx = property(
        lambda self: 0, lambda self, v: None
    )


_SERIAL_ENGINES = {"EngineType.DVE", "EngineType.Activation", "EngineType.Pool"}


def _join(a, b):
    for k, v in b.items():
        if a.get(k, -1) < v:
            a[k] = v
    return a


def fix_waits(nc, dma_deps=None, verbose=False):
    """Drop sync waits provably implied transitively and lower/drop the Tile
    DMA-lane serialization waits using the kernel's true-DMA-dependency
    annotations; assert the walrus per-instruction wait limits.

    Assumptions: DVE/ACT/Pool retire instruction N before dispatching N+1
    (drain semantics); an instruction dispatches only after its waits held
    and its same-engine predecessor dispatched; a DMA's remaining hazards
    are covered by its other waits when its lane wait is lowered to its
    annotated DMA-dependencies."""
    import concourse.mybir as mybir

    dma_deps = dma_deps or {}
    n_dropped = n_lowered = 0
    violations = []
    for fn in nc.m.functions:
        insts = []
        for blk in fn.blocks:
            insts.extend(blk.instructions)
        bad_sems = set()
        for inst in insts:
            si = inst.sync_info
            if si is None:
                continue
            for u in si.on_update:
                if u.update_mode not in ("sem-inc", "sem-add-imm"):
                    bad_sems.add(u.ant_name)
            for w in si.on_wait:
                if w.wait_mode != "sem-ge-imm":
                    bad_sems.add(w.ant_name)

        lane_cum = {}
        lane_counter = {}
        for inst in insts:
            si = inst.sync_info
            if si is None or "DMA" not in type(inst).__name__:
                continue
            for u in si.on_update:
                if u.update_mode in ("sem-inc", "sem-add-imm") and (
                    u.ant_name.startswith("DMAHW")
                    or u.ant_name.startswith("DMASW")
                ):
                    c = lane_counter.get(u.ant_name, 0) + (u.update_value or 16)
                    lane_counter[u.ant_name] = c
                    lane_cum[inst.name] = (u.ant_name, c)

        sem_hist = {}
        eng_know = {}
        eng_count = {}

        def lookup(sem, v):
            k = {sem: v}
            for cum, clk in sem_hist.get(sem, []):
                _join(k, clk)
                if cum >= v:
                    break
            return k

        for inst in insts:
            si = inst.sync_info
            tname = type(inst).__name__
            eng = str(getattr(inst, "engine", None))
            k0 = dict(eng_know.get(eng, {}))
            if eng in _SERIAL_ENGINES:
                for s, c in eng_count.items():
                    if s.startswith(eng):
                        _join(k0, {s.split("|", 1)[1]: c})
            if si is None:
                eng_know[eng] = k0
                continue
            waits = list(si.on_wait)
            is_dma = "DMA" in tname

            lowered = False
            if is_dma and inst.name in lane_cum:
                lane_sem, _my_cum = lane_cum[inst.name]
                need = 0
                for dep in dma_deps.get(inst.name, ()):
                    if dep not in lane_cum:
                        raise RuntimeError(
                            f"waitfix: unknown dma dep {dep} of {inst.name}"
                        )
                    dsem, dcum = lane_cum[dep]
                    if dsem != lane_sem:
                        raise RuntimeError(
                            f"waitfix: cross-lane dep {dep}->{inst.name}"
                        )
                    need = max(need, dcum)
                for i, w in enumerate(waits):
                    if w.ant_name == lane_sem and w.wait_mode == "sem-ge-imm":
                        if need == 0:
                            waits = waits[:i] + waits[i + 1 :]
                            lowered = True
                        elif w.wait_value > need:
                            nw = mybir.SyncWait(
                                sync_type=w.sync_type,
                                id=w.id,
                                ant_name=w.ant_name,
                                wait_mode=w.wait_mode,
                                wait_value=need,
                                wait_reg=w.wait_reg,
                            )
                            waits = waits[:i] + [nw] + waits[i + 1 :]
                            lowered = True
                        break
                if lowered:
                    n_lowered += 1

            looks = {}
            for i, w in enumerate(waits):
                if w.wait_mode == "sem-ge-imm" and w.ant_name not in bad_sems:
                    looks[i] = lookup(w.ant_name, w.wait_value)
            kept = set(range(len(waits)))
            changed = True
            while changed:
                changed = False
                for i in sorted(kept):
                    if i not in looks:
                        continue
                    w = waits[i]
                    cl = dict(k0)
                    for j in kept:
                        if j == i or j not in looks:
                            continue
                        _join(cl, looks[j])
                    if cl.get(w.ant_name, -1) >= w.wait_value:
                        kept.discard(i)
                        changed = True
            kd = k0
            for i, lk in looks.items():
                _join(kd, lk)
            eng_know[eng] = kd

            for u in si.on_update:
                if u.update_mode in ("sem-inc", "sem-add-imm"):
                    s = u.ant_name
                    if s in bad_sems:
                        continue
                    hist = sem_hist.setdefault(s, [])
                    prev = hist[-1][0] if hist else 0
                    hist.append((prev + (u.update_value or 1), dict(kd)))
                    if not (s.startswith("DMAHW") or s.startswith("DMASW")):
                        key = f"{eng}|{s}"
                        eng_count[key] = eng_count.get(key, 0) + (
                            u.update_value or 1
                        )

            if len(kept) < len(waits) or lowered:
                n_dropped += len(waits) - len(kept)
                new_waits = [waits[i] for i in sorted(kept)]
                inst.sync_info = mybir.SyncInfo(
                    on_wait=new_waits, on_update=list(si.on_update)
                )
            limit = 1 if is_dma else 2
            if len(kept) > limit:
                violations.append(
                    (
                        inst.name,
                        tname,
                        eng,
                        [
                            (waits[i].ant_name, waits[i].wait_value)
                            for i in sorted(kept)
                        ],
                    )
                )
    if verbose:
        print(f"waitfix: dropped {n_dropped}, lowered {n_lowered} waits")
        for v in violations:
            print("  VIOLATION:", v)
    if violations:
        raise RuntimeError(
            f"waitfix: {len(violations)} instructions exceed walrus sync-wait "
            f"limits: {violations[:8]}"
        )


B, C, H, W = 32, 512, 56, 56
S = H * W
G = C // 128
CR = 32
N_CORES = 8
B_LOC = B // N_CORES
Q = 112
NJ = S // Q  # 28 pixel chunks of 112 (3136 = 28*112)


def build_nc(repeat=1):
    pin_dma_lanes()
    import concourse.bass as bass
    import concourse.mybir as mybir
    import concourse.tile as tile

    f32 = mybir.dt.float32
    ts = bass.ts
    AF = mybir.ActivationFunctionType
    OP = mybir.AluOpType

    nc = bass.Bass("TRN2")
    x_in = nc.dram_tensor("x", [B_LOC, C, S], f32, kind="ExternalInput")
    w1t_avg = nc.dram_tensor("w1t_avg", [128, G, CR], f32, kind="ExternalInput")
    w1t_max = nc.dram_tensor("w1t_max", [128, G, CR], f32, kind="ExternalInput")
    w2t = nc.dram_tensor("w2t", [CR, G, 128], f32, kind="ExternalInput")
    b1d = nc.dram_tensor("b1c", [CR, 1], f32, kind="ExternalInput")
    b2x2 = nc.dram_tensor("b2x2", [128, G], f32, kind="ExternalInput")
    wbd = nc.dram_tensor("wb", [60, 10, 56], f32, kind="ExternalInput")
    b3d = nc.dram_tensor("b3c", [56, 1], f32, kind="ExternalInput")
    onesd = nc.dram_tensor("ones", [128, 1], f32, kind="ExternalInput")
    identd = nc.dram_tensor("ident", [128, 128], f32, kind="ExternalInput")

    out_d = nc.dram_tensor("out", [B_LOC, C, S], f32, kind="ExternalOutput")
    ca_d = nc.dram_tensor("ca", [B_LOC, C], f32, kind="ExternalOutput")
    sp_d = nc.dram_tensor("sp", [B_LOC, S], f32, kind="ExternalOutput")

    mrt = nc.dram_tensor("mrt", [2, 2, S], f32, kind="Internal")

    dma_deps = {}

    def dma(dst, src, deps=()):
        inst = nc.sync.dma_start(out=dst, in_=src)
        dma_deps[inst.ins.name] = [d for d in deps if d is not None]
        return inst.ins.name

    with tile.TileContext(nc) as tc:
        with (
            tc.tile_pool(name="consts", bufs=1) as consts,
            tc.tile_pool(name="big", bufs=1) as big,
            tc.tile_pool(name="work", bufs=1) as work,
            tc.tile_pool(name="ps", bufs=1, space="PSUM") as psp,
        ):
            # ---- constants ----
            w1a_sb = consts.tile([128, G, CR], f32, name="w1a_sb")
            w1m_sb = consts.tile([128, G, CR], f32, name="w1m_sb")
            w2t_sb = consts.tile([CR, G, 128], f32, name="w2t_sb")
            b1_sb = consts.tile([CR, 1], f32, name="b1_sb")
            b2_sb = consts.tile([128, G], f32, name="b2_sb")
            wb_sb = consts.tile([60, 10, 56], f32, name="wb_sb")
            b3_sb = consts.tile([56, 1], f32, name="b3_sb")
            ones_sb = consts.tile([128, 1], f32, name="ones_sb")
            id_sb = consts.tile([128, 128], f32, name="id_sb")
            caT = consts.tile([128, G, B_LOC], f32, name="caT")
            for dst, src in (
                (w1a_sb, w1t_avg),
                (w1m_sb, w1t_max),
                (w2t_sb, w2t),
                (b1_sb, b1d),
                (b2_sb, b2x2),
                (wb_sb, wbd),
                (b3_sb, b3d),
                (ones_sb, onesd),
                (id_sb, identd),
            ):
                dma(dst, src[:])

            # ---- per-parity working tiles ----
            xx = [big.tile([128, G, S], f32, name=f"xx{p}") for p in range(2)]
            mt1 = [big.tile([128, S], f32, name=f"mt1_{p}") for p in range(2)]
            mt2 = [big.tile([128, S], f32, name=f"mt2_{p}") for p in range(2)]
            spb = [big.tile([128, S], f32, name=f"spb{p}") for p in range(2)]
            pooled = [work.tile([128, G, 2], f32, name=f"pooled{p}") for p in range(2)]
            h_sb = [work.tile([CR, 2], f32, name=f"h_sb{p}") for p in range(2)]
            hsum = [work.tile([CR, 1], f32, name=f"hsum{p}") for p in range(2)]
            pixmax = [work.tile([Q, NJ], f32, name=f"pixmax{p}") for p in range(2)]
            pm_sb = [work.tile([Q, NJ], f32, name=f"pm_sb{p}") for p in range(2)]
            mpad = [work.tile([60, 2, 60], f32, name=f"mpad{p}") for p in range(2)]
            sp_sb = [work.tile([56, 56], f32, name=f"sp_sb{p}") for p in range(2)]
            for p in range(2):
                nc.vector.memset(mpad[p], 0.0)

            st_out = {}
            ld_pad = {}

            for it in range(B_LOC * repeat):
                b = it % B_LOC
                p = it % 2
                xr = x_in[b].rearrange("(g q) s -> q g s", q=128)
                ld_x = dma(xx[p], xr, deps=[st_out.get(it - 2)])

                # ---- pooled stats: sum on ACT (accum_out), max on DVE ----
                for g in range(G):
                    nc.scalar.activation(
                        out=mt1[p], in_=xx[p][:, g, :],
                        func=AF.Identity, bias=0.0, scale=1.0,
                        accum_out=pooled[p][:, g, 0:1],
                    )
                    nc.vector.reduce_max(
                        out=pooled[p][:, g, 1:2], in_=xx[p][:, g, :],
                        axis=mybir.AxisListType.X,
                    )

                # ---- MLP ----
                ps_h = psp.tile([CR, 2], f32, name=f"ps_h_{it}", tag="ps_mlp", bufs=2)
                for col, wt in ((0, w1a_sb), (1, w1m_sb)):
                    for g in range(G):
                        nc.tensor.matmul(
                            ps_h[:, col : col + 1],
                            wt[:, g, :],
                            pooled[p][:, g, col : col + 1],
                            start=(g == 0),
                            stop=(g == G - 1),
                        )
                nc.scalar.activation(
                    out=h_sb[p], in_=ps_h, func=AF.Relu, bias=b1_sb, scale=1.0
                )
                nc.vector.tensor_tensor(
                    out=hsum[p], in0=h_sb[p][:, 0:1], in1=h_sb[p][:, 1:2],
                    op=OP.add,
                )
                ps_ca = psp.tile([128, G], f32, name=f"ps_ca_{it}", tag="ps_mlp", bufs=2)
                for g in range(G):
                    nc.tensor.matmul(
                        ps_ca[:, g : g + 1], w2t_sb[:, g, :], hsum[p],
                        start=True, stop=True,
                    )
                for g in range(G):
                    nc.scalar.activation(
                        out=caT[:, g, b : b + 1], in_=ps_ca[:, g : g + 1],
                        func=AF.Sigmoid, bias=b2_sb[:, g : g + 1], scale=1.0,
                    )

                # ---- xg = x * ca (in place, on ACT: frees DVE) ----
                for g in range(G):
                    nc.scalar.mul(
                        out=xx[p][:, g, :], in_=xx[p][:, g, :],
                        mul=caT[:, g, b : b + 1],
                    )

                # ---- channel mean map (PE): psum[q, j] = sum_c xg[c, j*128+q] ----
                ps_pm = psp.tile([Q, NJ], f32, name=f"ps_pm_{it}", tag="ps_pmcv", bufs=2)
                for g in range(G):
                    for j in range(NJ):
                        nc.tensor.matmul(
                            ps_pm[:, j : j + 1],
                            xx[p][:, g, ts(j, Q)],
                            ones_sb,
                            start=(g == 0),
                            stop=(g == G - 1),
                        )
                nc.vector.tensor_copy(out=pm_sb[p], in_=ps_pm)

                # ---- channel max map: group tree then PE transpose + reduce ----
                nc.vector.tensor_tensor(
                    out=mt1[p], in0=xx[p][:, 0, :], in1=xx[p][:, 1, :], op=OP.max
                )
                nc.vector.tensor_tensor(
                    out=mt2[p], in0=xx[p][:, 2, :], in1=xx[p][:, 3, :], op=OP.max
                )
                nc.vector.tensor_tensor(
                    out=mt1[p], in0=mt1[p], in1=mt2[p], op=OP.max
                )
                for jj in range((NJ + 3) // 4):
                    qc = min(4, NJ - jj * 4)
                    ps_tr = psp.tile(
                        [Q, 4, 128], f32, name=f"ps_tr_{it}_{jj}", tag="ps_tr",
                        bufs=2,
                    )
                    for q in range(qc):
                        nc.tensor.transpose(
                            ps_tr[:, q, :], mt1[p][:, ts(jj * 4 + q, Q)], id_sb
                        )
                    nc.vector.reduce_max(
                        out=pixmax[p][:, jj * 4 : jj * 4 + qc],
                        in_=ps_tr[:, :qc, :],
                        axis=mybir.AxisListType.X,
                    )

                # ---- maps -> DRAM -> padded tile ----
                st_m = dma(
                    mrt[p, 0].rearrange("(j q) -> q j", j=NJ, q=Q), pm_sb[p],
                    deps=[ld_pad.get(it - 2)],
                )
                st_x = dma(
                    mrt[p, 1].rearrange("(j q) -> q j", j=NJ, q=Q), pixmax[p],
                    deps=[ld_pad.get(it - 2)],
                )
                ld_pad[it] = dma(
                    mpad[p][2:58, :, 2:58],
                    mrt[p].rearrange("c (h w) -> h c w", w=56),
                    deps=[st_m, st_x],
                )

                # ---- 5x5 conv via banded matmuls ----
                ps_cv = psp.tile([56, 56], f32, name=f"ps_cv_{it}", tag="ps_pmcv", bufs=2)
                first = True
                for c in range(2):
                    for dx in range(5):
                        nc.tensor.matmul(
                            ps_cv,
                            wb_sb[:, c * 5 + dx, :],
                            mpad[p][:, c, dx : dx + 56],
                            start=first,
                            stop=(c == 1 and dx == 4),
                        )
                        first = False
                nc.scalar.activation(
                    out=sp_sb[p], in_=ps_cv, func=AF.Sigmoid, bias=b3_sb, scale=1.0
                )
                st_sp = dma(
                    sp_d[b].rearrange("(h w) -> h w", w=56), sp_sb[p], deps=[]
                )
                ld_spb = dma(
                    spb[p], sp_d[b].partition_broadcast(128), deps=[st_sp]
                )

                # ---- out = xg * sp ----
                for g in range(G):
                    nc.vector.tensor_tensor(
                        out=xx[p][:, g, :], in0=xx[p][:, g, :], in1=spb[p],
                        op=OP.mult,
                    )
                st_out[it] = dma(
                    out_d[b].rearrange("(g q) s -> q g s", q=128), xx[p], deps=[]
                )

            for g in range(G):
                dma(
                    ca_d[:, g * 128 : (g + 1) * 128].rearrange("b q -> q b"),
                    caT[:, g, :],
                    deps=[],
                )

    fix_waits(nc, dma_deps, verbose=False)
    return nc


def _host_prep(w1, b1, w2, b2, w3, b3):
    w1 = np.asarray(w1, np.float32)
    w2 = np.asarray(w2, np.float32)
    w3 = np.asarray(w3, np.float32)
    w1t = np.transpose(w1).reshape(G, 128, CR).transpose(1, 0, 2).copy()
    w1t_avg = (w1t / float(S)).astype(np.float32)
    w1t_max = w1t.astype(np.float32)
    w2t = np.transpose(w2).reshape(CR, G, 128).copy().astype(np.float32)
    b1c = np.asarray(b1, np.float32).reshape(CR, 1).copy()
    b2x2 = (2.0 * np.asarray(b2, np.float32)).reshape(G, 128).T.copy()
    wb = np.zeros((60, 10, 56), np.float32)
    for c in range(2):
        scale = (1.0 / C) if c == 0 else 1.0
        for dx in range(5):
            for dy in range(5):
                for h in range(56):
                    wb[h + dy, c * 5 + dx, h] = w3[0, c, dy, dx] * scale
    b3c = np.full((56, 1), np.asarray(b3, np.float32).reshape(-1)[0], np.float32)
    ones = np.ones((128, 1), np.float32)
    ident = np.eye(128, dtype=np.float32)
    return dict(
        w1t_avg=w1t_avg, w1t_max=w1t_max, w2t=w2t, b1c=b1c, b2x2=b2x2,
        wb=wb, b3c=b3c, ones=ones, ident=ident,
    )


_cached_nc = None


def kernel(x, w1, b1, w2, b2, w3, b3):
    global _cached_nc
    from concourse.bass_utils import run_bass_kernel_spmd

    x = np.ascontiguousarray(np.asarray(x, np.float32))
    params = _host_prep(w1, b1, w2, b2, w3, b3)
    if _cached_nc is None:
        _cached_nc = build_nc()
    nc = _cached_nc
    in_maps = []
    for i in range(N_CORES):
        m = {"x": x[i * B_LOC : (i + 1) * B_LOC].reshape(B_LOC, C, S)}
        m.update(params)
        in_maps.append(m)
    res = run_bass_kernel_spmd(nc, in_maps, core_ids=list(range(N_CORES)))
    out = np.concatenate(
        [r["out"].reshape(B_LOC, C, H, W) for r in res.results], axis=0
    )
    ca = np.concatenate(
        [r["ca"].reshape(B_LOC, C, 1, 1) for r in res.results], axis=0
    )
    sp = np.concatenate(
        [r["sp"].reshape(B_LOC, 1, H, W) for r in res.results], axis=0
    )
    return (out, (ca, sp))


# revision 8
# speedup vs baseline: 4.0824x; 1.6556x over previous
"""CBAM-style channel+spatial attention block (nn_CBAModule) on 8 TRN2
NeuronCores, pure data-parallel over the batch dimension.

Self-contained: builds a Bass/Tile kernel per 4-image shard, runs it SPMD on
cores 0-7 via concourse.bass_utils.run_bass_kernel_spmd, gathers the full
outputs.  All weight reshaping/transposition is done host-side in numpy and
fed as extra kernel inputs.

The pinned walrus codegen accepts at most 1 sync-wait on DMA
pseudo-instructions and 2 on everything else, while Tile emits transitively
redundant waits; fix_waits() below prunes them (see its docstring for the
soundness argument).
"""

import sys

for _p in ("/opt/trn_rl_repo",):
    if _p not in sys.path:
        sys.path.insert(0, _p)

import numpy as np

import concourse.tile_sem_assignment as _tsa

_pinned = False


def pin_dma_lanes():
    """Force every DMA onto one Tile completion-sem lane (must run before
    TileContext is created).  All DMAs are issued from the single SP HWDGE
    ring, so single-lane cumulative thresholds stay sound: every DMA adds
    exactly 16 increments, hence observing lane >= 16*k where k DMAs were
    issued earlier guarantees all k completed regardless of completion
    order."""
    global _pinned
    if _pinned:
        return
    _pinned = True
    orig = _tsa.TileClockTick.__init__

    def patched(self, *a, **k):
        orig(self, *a, **k)
        self.swdge_sem_count = 1

    _tsa.TileClockTick.__init__ = patched
    _tsa.TileClockTick.next_hw_dma_idx = property(
        lambda self: 0, lambda self, v: None
    )


_SERIAL_ENGINES = {"EngineType.DVE", "EngineType.Activation", "EngineType.Pool"}


def _join(a, b):
    for k, v in b.items():
        if a.get(k, -1) < v:
            a[k] = v
    return a


def fix_waits(nc, dma_deps=None, verbose=False):
    """Drop sync waits provably implied transitively and lower/drop the Tile
    DMA-lane serialization waits using the kernel's true-DMA-dependency
    annotations; assert the walrus per-instruction wait limits.

    Assumptions: DVE/ACT/Pool retire instruction N before dispatching N+1
    (drain semantics); an instruction dispatches only after its waits held
    and its same-engine predecessor dispatched; a DMA's remaining hazards
    are covered by its other waits when its lane wait is lowered to its
    annotated DMA-dependencies."""
    import concourse.mybir as mybir

    dma_deps = dma_deps or {}
    n_dropped = n_lowered = 0
    violations = []
    for fn in nc.m.functions:
        insts = []
        for blk in fn.blocks:
            insts.extend(blk.instructions)
        bad_sems = set()
        for inst in insts:
            si = inst.sync_info
            if si is None:
                continue
            for u in si.on_update:
                if u.update_mode not in ("sem-inc", "sem-add-imm"):
                    bad_sems.add(u.ant_name)
            for w in si.on_wait:
                if w.wait_mode != "sem-ge-imm":
                    bad_sems.add(w.ant_name)

        lane_cum = {}
        lane_counter = {}
        for inst in insts:
            si = inst.sync_info
            if si is None or "DMA" not in type(inst).__name__:
                continue
            for u in si.on_update:
                if u.update_mode in ("sem-inc", "sem-add-imm") and (
                    u.ant_name.startswith("DMAHW")
                    or u.ant_name.startswith("DMASW")
                ):
                    c = lane_counter.get(u.ant_name, 0) + (u.update_value or 16)
                    lane_counter[u.ant_name] = c
                    lane_cum[inst.name] = (u.ant_name, c)

        sem_hist = {}
        eng_know = {}
        eng_count = {}

        def lookup(sem, v):
            k = {sem: v}
            for cum, clk in sem_hist.get(sem, []):
                _join(k, clk)
                if cum >= v:
                    break
            return k

        for inst in insts:
            si = inst.sync_info
            tname = type(inst).__name__
            eng = str(getattr(inst, "engine", None))
            k0 = dict(eng_know.get(eng, {}))
            if eng in _SERIAL_ENGINES:
                for s, c in eng_count.items():
                    if s.startswith(eng):
                        _join(k0, {s.split("|", 1)[1]: c})
            if si is None:
                eng_know[eng] = k0
                continue
            waits = list(si.on_wait)
            is_dma = "DMA" in tname

            lowered = False
            if is_dma and inst.name in lane_cum:
                lane_sem, _my_cum = lane_cum[inst.name]
                need = 0
                for dep in dma_deps.get(inst.name, ()):
                    if dep not in lane_cum:
                        raise RuntimeError(
                            f"waitfix: unknown dma dep {dep} of {inst.name}"
                        )
                    dsem, dcum = lane_cum[dep]
                    if dsem != lane_sem:
                        raise RuntimeError(
                            f"waitfix: cross-lane dep {dep}->{inst.name}"
                        )
                    need = max(need, dcum)
                for i, w in enumerate(waits):
                    if w.ant_name == lane_sem and w.wait_mode == "sem-ge-imm":
                        if need == 0:
                            waits = waits[:i] + waits[i + 1 :]
                            lowered = True
                        elif w.wait_value > need:
                            nw = mybir.SyncWait(
                                sync_type=w.sync_type,
                                id=w.id,
                                ant_name=w.ant_name,
                                wait_mode=w.wait_mode,
                                wait_value=need,
                                wait_reg=w.wait_reg,
                            )
                            waits = waits[:i] + [nw] + waits[i + 1 :]
                            lowered = True
                        break
                if lowered:
                    n_lowered += 1

            looks = {}
            for i, w in enumerate(waits):
                if w.wait_mode == "sem-ge-imm" and w.ant_name not in bad_sems:
                    looks[i] = lookup(w.ant_name, w.wait_value)
            kept = set(range(len(waits)))
            changed = True
            while changed:
                changed = False
                for i in sorted(kept):
                    if i not in looks:
                        continue
                    w = waits[i]
                    cl = dict(k0)
                    for j in kept:
                        if j == i or j not in looks:
                            continue
                        _join(cl, looks[j])
                    if cl.get(w.ant_name, -1) >= w.wait_value:
                        kept.discard(i)
                        changed = True
            kd = k0
            for i, lk in looks.items():
                _join(kd, lk)
            eng_know[eng] = kd

            for u in si.on_update:
                if u.update_mode in ("sem-inc", "sem-add-imm"):
                    s = u.ant_name
                    if s in bad_sems:
                        continue
                    hist = sem_hist.setdefault(s, [])
                    prev = hist[-1][0] if hist else 0
                    hist.append((prev + (u.update_value or 1), dict(kd)))
                    if not (s.startswith("DMAHW") or s.startswith("DMASW")):
                        key = f"{eng}|{s}"
                        eng_count[key] = eng_count.get(key, 0) + (
                            u.update_value or 1
                        )

            if len(kept) < len(waits) or lowered:
                n_dropped += len(waits) - len(kept)
                new_waits = [waits[i] for i in sorted(kept)]
                inst.sync_info = mybir.SyncInfo(
                    on_wait=new_waits, on_update=list(si.on_update)
                )
            limit = 1 if is_dma else 2
            if len(kept) > limit:
                violations.append(
                    (
                        inst.name,
                        tname,
                        eng,
                        [
                            (waits[i].ant_name, waits[i].wait_value)
                            for i in sorted(kept)
                        ],
                    )
                )
    if verbose:
        print(f"waitfix: dropped {n_dropped}, lowered {n_lowered} waits")
        for v in violations:
            print("  VIOLATION:", v)
    if violations:
        raise RuntimeError(
            f"waitfix: {len(violations)} instructions exceed walrus sync-wait "
            f"limits: {violations[:8]}"
        )


B, C, H, W = 32, 512, 56, 56
S = H * W
G = C // 128
CR = 32
N_CORES = 8
B_LOC = B // N_CORES
Q = 112
NJ = S // Q  # 28 pixel chunks of 112 (3136 = 28*112)


def build_nc(repeat=1):
    pin_dma_lanes()
    import concourse.bass as bass
    import concourse.mybir as mybir
    import concourse.tile as tile

    f32 = mybir.dt.float32
    ts = bass.ts
    AF = mybir.ActivationFunctionType
    OP = mybir.AluOpType

    nc = bass.Bass("TRN2")
    x_in = nc.dram_tensor("x", [B_LOC, C, S], f32, kind="ExternalInput")
    w1t_avg = nc.dram_tensor("w1t_avg", [128, G, CR], f32, kind="ExternalInput")
    w1t_max = nc.dram_tensor("w1t_max", [128, G, CR], f32, kind="ExternalInput")
    w2t = nc.dram_tensor("w2t", [CR, G, 128], f32, kind="ExternalInput")
    b1d = nc.dram_tensor("b1c", [CR, 1], f32, kind="ExternalInput")
    b2x2 = nc.dram_tensor("b2x2", [128, G], f32, kind="ExternalInput")
    wbd = nc.dram_tensor("wb", [60, 10, 56], f32, kind="ExternalInput")
    b3d = nc.dram_tensor("b3c", [56, 1], f32, kind="ExternalInput")
    onesd = nc.dram_tensor("ones", [128, 1], f32, kind="ExternalInput")
    identd = nc.dram_tensor("ident", [128, 128], f32, kind="ExternalInput")

    out_d = nc.dram_tensor("out", [B_LOC, C, S], f32, kind="ExternalOutput")
    ca_d = nc.dram_tensor("ca", [B_LOC, C], f32, kind="ExternalOutput")
    sp_d = nc.dram_tensor("sp", [B_LOC, S], f32, kind="ExternalOutput")

    mrt = nc.dram_tensor("mrt", [2, 2, S], f32, kind="Internal")

    dma_deps = {}

    def dma(dst, src, deps=()):
        inst = nc.sync.dma_start(out=dst, in_=src)
        dma_deps[inst.ins.name] = [d for d in deps if d is not None]
        return inst.ins.name

    with tile.TileContext(nc) as tc:
        with (
            tc.tile_pool(name="consts", bufs=1) as consts,
            tc.tile_pool(name="big", bufs=1) as big,
            tc.tile_pool(name="work", bufs=1) as work,
            tc.tile_pool(name="ps", bufs=1, space="PSUM") as psp,
        ):
            # ---- constants ----
            w1a_sb = consts.tile([128, G, CR], f32, name="w1a_sb")
            w1m_sb = consts.tile([128, G, CR], f32, name="w1m_sb")
            w2t_sb = consts.tile([CR, G, 128], f32, name="w2t_sb")
            b1_sb = consts.tile([CR, 1], f32, name="b1_sb")
            b2_sb = consts.tile([128, G], f32, name="b2_sb")
            wb_sb = consts.tile([60, 10, 56], f32, name="wb_sb")
            b3_sb = consts.tile([56, 1], f32, name="b3_sb")
            ones_sb = consts.tile([128, 1], f32, name="ones_sb")
            id_sb = consts.tile([128, 128], f32, name="id_sb")
            caT = consts.tile([128, G, B_LOC], f32, name="caT")
            for dst, src in (
                (w1a_sb, w1t_avg),
                (w1m_sb, w1t_max),
                (w2t_sb, w2t),
                (b1_sb, b1d),
                (b2_sb, b2x2),
                (wb_sb, wbd),
                (b3_sb, b3d),
                (ones_sb, onesd),
                (id_sb, identd),
            ):
                dma(dst, src[:])

            # ---- per-parity working tiles ----
            xx = [big.tile([128, G, S], f32, name=f"xx{p}") for p in range(2)]
            mt1 = [big.tile([128, S], f32, name=f"mt1_{p}") for p in range(2)]
            mt2 = [big.tile([128, S], f32, name=f"mt2_{p}") for p in range(2)]
            spb = [big.tile([128, S], f32, name=f"spb{p}") for p in range(2)]
            pooled = [work.tile([128, G, 2], f32, name=f"pooled{p}") for p in range(2)]
            h_sb = [work.tile([CR, 2], f32, name=f"h_sb{p}") for p in range(2)]
            hsum = [work.tile([CR, 1], f32, name=f"hsum{p}") for p in range(2)]
            pixmax = [work.tile([Q, NJ], f32, name=f"pixmax{p}") for p in range(2)]
            pm_sb = [work.tile([Q, NJ], f32, name=f"pm_sb{p}") for p in range(2)]
            mpad = [work.tile([60, 2, 60], f32, name=f"mpad{p}") for p in range(2)]
            sp_sb = [work.tile([56, 56], f32, name=f"sp_sb{p}") for p in range(2)]
            for p in range(2):
                nc.vector.memset(mpad[p], 0.0)

            st_out = {}
            ld_pad = {}

            for it in range(B_LOC * repeat):
                b = it % B_LOC
                p = it % 2
                xr = x_in[b].rearrange("(g q) s -> q g s", q=128)
                prev_sts = st_out.get(it - 2) or [None] * G
                for g in range(G):
                    dma(xx[p][:, g, :], xr[:, g, :], deps=[prev_sts[g]])

                # ---- pooled stats: sum on ACT (accum_out), max on DVE ----
                for g in range(G):
                    nc.scalar.activation(
                        out=mt1[p], in_=xx[p][:, g, :],
                        func=AF.Identity, bias=0.0, scale=1.0,
                        accum_out=pooled[p][:, g, 0:1],
                    )
                    nc.vector.reduce_max(
                        out=pooled[p][:, g, 1:2], in_=xx[p][:, g, :],
                        axis=mybir.AxisListType.X,
                    )

                # ---- MLP ----
                ps_h = psp.tile([CR, 2], f32, name=f"ps_h_{it}", tag="ps_mlp", bufs=2)
                for col, wt in ((0, w1a_sb), (1, w1m_sb)):
                    for g in range(G):
                        nc.tensor.matmul(
                            ps_h[:, col : col + 1],
                            wt[:, g, :],
                            pooled[p][:, g, col : col + 1],
                            start=(g == 0),
                            stop=(g == G - 1),
                        )
                nc.scalar.activation(
                    out=h_sb[p], in_=ps_h, func=AF.Relu, bias=b1_sb, scale=1.0
                )
                nc.vector.tensor_tensor(
                    out=hsum[p], in0=h_sb[p][:, 0:1], in1=h_sb[p][:, 1:2],
                    op=OP.add,
                )
                ps_ca = psp.tile([128, G], f32, name=f"ps_ca_{it}", tag="ps_mlp", bufs=2)
                for g in range(G):
                    nc.tensor.matmul(
                        ps_ca[:, g : g + 1], w2t_sb[:, g, :], hsum[p],
                        start=True, stop=True,
                    )
                for g in range(G):
                    nc.scalar.activation(
                        out=caT[:, g, b : b + 1], in_=ps_ca[:, g : g + 1],
                        func=AF.Sigmoid, bias=b2_sb[:, g : g + 1], scale=1.0,
                    )

                # ---- xg = x * ca (in place, on ACT: frees DVE) ----
                for g in range(G):
                    nc.scalar.mul(
                        out=xx[p][:, g, :], in_=xx[p][:, g, :],
                        mul=caT[:, g, b : b + 1],
                    )

                # ---- channel mean map (PE): psum[q, j] = sum_c xg[c, j*128+q] ----
                ps_pm = psp.tile([Q, NJ], f32, name=f"ps_pm_{it}", tag="ps_pmcv", bufs=2)
                for g in range(G):
                    for j in range(NJ):
                        nc.tensor.matmul(
                            ps_pm[:, j : j + 1],
                            xx[p][:, g, ts(j, Q)],
                            ones_sb,
                            start=(g == 0),
                            stop=(g == G - 1),
                        )
                nc.vector.tensor_copy(out=pm_sb[p], in_=ps_pm)

                # ---- channel max map: group tree then PE transpose + reduce ----
                nc.vector.tensor_tensor(
                    out=mt1[p], in0=xx[p][:, 0, :], in1=xx[p][:, 1, :], op=OP.max
                )
                nc.vector.tensor_tensor(
                    out=mt2[p], in0=xx[p][:, 2, :], in1=xx[p][:, 3, :], op=OP.max
                )
                nc.vector.tensor_tensor(
                    out=mt1[p], in0=mt1[p], in1=mt2[p], op=OP.max
                )
                for jj in range((NJ + 3) // 4):
                    qc = min(4, NJ - jj * 4)
                    ps_tr = psp.tile(
                        [Q, 4, 128], f32, name=f"ps_tr_{it}_{jj}", tag="ps_tr",
                        bufs=2,
                    )
                    for q in range(qc):
                        nc.tensor.transpose(
                            ps_tr[:, q, :], mt1[p][:, ts(jj * 4 + q, Q)], id_sb
                        )
                    nc.vector.reduce_max(
                        out=pixmax[p][:, jj * 4 : jj * 4 + qc],
                        in_=ps_tr[:, :qc, :],
                        axis=mybir.AxisListType.X,
                    )

                # ---- maps -> DRAM -> padded tile ----
                st_m = dma(
                    mrt[p, 0].rearrange("(j q) -> q j", j=NJ, q=Q), pm_sb[p],
                    deps=[ld_pad.get(it - 2)],
                )
                st_x = dma(
                    mrt[p, 1].rearrange("(j q) -> q j", j=NJ, q=Q), pixmax[p],
                    deps=[ld_pad.get(it - 2)],
                )
                ld_pad[it] = dma(
                    mpad[p][2:58, :, 2:58],
                    mrt[p].rearrange("c (h w) -> h c w", w=56),
                    deps=[st_m, st_x],
                )

                # ---- 5x5 conv via banded matmuls ----
                ps_cv = psp.tile([56, 56], f32, name=f"ps_cv_{it}", tag="ps_pmcv", bufs=2)
                first = True
                for c in range(2):
                    for dx in range(5):
                        nc.tensor.matmul(
                            ps_cv,
                            wb_sb[:, c * 5 + dx, :],
                            mpad[p][:, c, dx : dx + 56],
                            start=first,
                            stop=(c == 1 and dx == 4),
                        )
                        first = False
                nc.scalar.activation(
                    out=sp_sb[p], in_=ps_cv, func=AF.Sigmoid, bias=b3_sb, scale=1.0
                )
                st_sp = dma(
                    sp_d[b].rearrange("(h w) -> h w", w=56), sp_sb[p], deps=[]
                )
                ld_spb = dma(
                    spb[p], sp_d[b].partition_broadcast(128), deps=[st_sp]
                )

                # ---- out = xg * sp ----
                for g in range(G):
                    nc.vector.tensor_tensor(
                        out=xx[p][:, g, :], in0=xx[p][:, g, :], in1=spb[p],
                        op=OP.mult,
                    )
                odr = out_d[b].rearrange("(g q) s -> q g s", q=128)
                st_out[it] = [
                    dma(odr[:, g, :], xx[p][:, g, :], deps=[]) for g in range(G)
                ]

            for g in range(G):
                dma(
                    ca_d[:, g * 128 : (g + 1) * 128].rearrange("b q -> q b"),
                    caT[:, g, :],
                    deps=[],
                )

    fix_waits(nc, dma_deps, verbose=False)
    return nc


def _host_prep(w1, b1, w2, b2, w3, b3):
    w1 = np.asarray(w1, np.float32)
    w2 = np.asarray(w2, np.float32)
    w3 = np.asarray(w3, np.float32)
    w1t = np.transpose(w1).reshape(G, 128, CR).transpose(1, 0, 2).copy()
    w1t_avg = (w1t / float(S)).astype(np.float32)
    w1t_max = w1t.astype(np.float32)
    w2t = np.transpose(w2).reshape(CR, G, 128).copy().astype(np.float32)
    b1c = np.asarray(b1, np.float32).reshape(CR, 1).copy()
    b2x2 = (2.0 * np.asarray(b2, np.float32)).reshape(G, 128).T.copy()
    wb = np.zeros((60, 10, 56), np.float32)
    for c in range(2):
        scale = (1.0 / C) if c == 0 else 1.0
        for dx in range(5):
            for dy in range(5):
                for h in range(56):
                    wb[h + dy, c * 5 + dx, h] = w3[0, c, dy, dx] * scale
    b3c = np.full((56, 1), np.asarray(b3, np.float32).reshape(-1)[0], np.float32)
    ones = np.ones((128, 1), np.float32)
    ident = np.eye(128, dtype=np.float32)
    return dict(
        w1t_avg=w1t_avg, w1t_max=w1t_max, w2t=w2t, b1c=b1c, b2x2=b2x2,
        wb=wb, b3c=b3c, ones=ones, ident=ident,
    )


_cached_nc = None


def kernel(x, w1, b1, w2, b2, w3, b3):
    global _cached_nc
    from concourse.bass_utils import run_bass_kernel_spmd

    x = np.ascontiguousarray(np.asarray(x, np.float32))
    params = _host_prep(w1, b1, w2, b2, w3, b3)
    if _cached_nc is None:
        _cached_nc = build_nc()
    nc = _cached_nc
    in_maps = []
    for i in range(N_CORES):
        m = {"x": x[i * B_LOC : (i + 1) * B_LOC].reshape(B_LOC, C, S)}
        m.update(params)
        in_maps.append(m)
    res = run_bass_kernel_spmd(nc, in_maps, core_ids=list(range(N_CORES)))
    out = np.concatenate(
        [r["out"].reshape(B_LOC, C, H, W) for r in res.results], axis=0
    )
    ca = np.concatenate(
        [r["ca"].reshape(B_LOC, C, 1, 1) for r in res.results], axis=0
    )
    sp = np.concatenate(
        [r["sp"].reshape(B_LOC, 1, H, W) for r in res.results], axis=0
    )
    return (out, (ca, sp))
